# revision 22
# baseline (speedup 1.0000x reference)
"""Trainium2 Bass kernel for nn_CESAR_24309514895978 (ragged_sequence).

Math (per batch b):
  m0 = (am==1)&(tt==0); m1 = (am==1)&(tt==1)
  score[i,j] = |emb_n[i] . emb_n[j]|   (L2-normalized embeddings)
  logits[i,j] = (emb@Wq.T+bq)[i] . (emb@Wk.T+bk)[j]
  cs[b] = sum_{valid ij} softmax_flat(logits | i in m0, j in m1)[i,j] * score[i,j]

Ragged compaction: only ~25% of tokens are in m0 and ~25% in m1, so the
host gathers the valid tokens and the device works on compacted panels:
rows = m1 tokens of 2 batches packed (<=2*128), cols = m0 tokens (free
dim ~260).  Matmul cost scales with the free dim only, so rows use the
partition dim (2 chunks) and cols the free dim.

Constant folding (host): logits = embaug_r @ A_aug @ embaug_c.T with
A_aug = [[Wk.T@Wq, Wk.T@bq], [bk.T@Wq, bq.bk]].  The device gets
  at  = (Wq.T@Wk)[db, da]  (stage-1 lhsT, bf16 to halve its DMA)
  ucol= Wk.T@bq            (bias riding the PSUM->SBUF copy of P)
  prow= emb_c@(Wq.T@bk)+bq.bk  (host-computed rank-1 row, in the mask mm)
Norms r=1/||emb|| are computed on the host; r_c rides a broadcast row,
r_r is applied host-side to the W partials.

Batch identity inside a packed panel is enforced with a K=4 mask matmul
(sum of non-positive rank-1 terms; no large-value cancellation):
  [ones, b0r, b1r, padr] x [prow, -1e30*b1c, -1e30*b0c, -1e30*ones]
No on-device max: exp uses a constant bias -M0 (uploaded, so a retry
with a larger M0 needs no recompile); W/Z ratio cancels the shift.

Device per core: stage1 P = at.T @ embc (64 mm), gram G = embr.T @ embc
(16 mm), stage2 L = embr.T @ paug + mask (18 mm); exp+accum -> Z rows,
stt(gw,E)+accum -> W rows.  Host: segment-sum rows by batch, cs = W/Z.
"""
import numpy as np
import ml_dtypes

import concourse.tile as tile
from concourse import bacc, mybir
from concourse.bass_utils import run_bass_kernel_spmd

B, S, D = 16, 512, 1024
NCORES = 8
NCH = D // 128             # 8 contraction chunks
NEG = np.float32(-1e30)
M0 = 60.0                  # logit shift; exp(L - M0), max logit ~58
EPS = 1e-12

F32 = mybir.dt.float32
F32R = mybir.dt.float32r
F16 = mybir.dt.float16    # 10-bit mantissa at bf16 cost: full-rate PE, 2B/elem
AFT = mybir.ActivationFunctionType
ALU = mybir.AluOpType

PROFILE = False            # set True (e.g. from test.py) to capture NTFF profile
LAST_RESULTS = None        # BassKernelResults of the last run (for test.py)

ST2_F32R = False           # stage2 (embr x paug) in f32r instead of f16

_builds = {}


def _to_fp32r(x: np.ndarray) -> np.ndarray:
    """Round fp32 -> fp32r encoding (RNE to 11 explicit mantissa bits)."""
    u = np.ascontiguousarray(x, dtype=np.float32).view(np.uint32).astype(np.uint64)
    u = (u + 0x7FF + ((u >> 12) & 1)) & np.uint64(0xFFFFF000)
    return u.astype(np.uint32).view(np.float32)


def _f16(x: np.ndarray) -> np.ndarray:
    return np.ascontiguousarray(np.asarray(x, np.float32)).astype(np.float16)


def _build(nr: int, c: int, st2_f32r: bool):
    key = (nr, c, st2_f32r)
    if key in _builds:
        return _builds[key]

    R = nr * 128
    nc = bacc.Bacc("TRN2", target_bir_lowering=False, debug=False)

    # emb panels are packed chunk-major along the free dim so one DMA moves
    # 4KB+ per partition line (per-partition descriptors stay efficient)
    at_d = nc.dram_tensor("at", [NCH, 128, D], F16, kind="ExternalInput").ap()
    embc_d = nc.dram_tensor("embc", [128, NCH * c], F16, kind="ExternalInput").ap()
    embr_d = nc.dram_tensor("embr", [128, NCH * R], F16, kind="ExternalInput").ap()
    if st2_f32r:
        embr32_d = nc.dram_tensor("embr32", [128, NCH * R], F32R,
                                  kind="ExternalInput").ap()
    mrows_d = nc.dram_tensor("mrows", [4, R], F32R, kind="ExternalInput").ap()
    rhs4_d = nc.dram_tensor("rhs4", [4, c], F32R, kind="ExternalInput").ap()
    ucol_d = nc.dram_tensor("ucol", [128, NCH], F32, kind="ExternalInput").ap()
    rrow_d = nc.dram_tensor("rrow", [1, c], F32, kind="ExternalInput").ap()
    m0col_d = nc.dram_tensor("m0col", [128, 1], F32, kind="ExternalInput").ap()

    zw_d = nc.dram_tensor("zw", [128, 2 * nr], F32, kind="ExternalOutput").ap()

    pdt = F32R if st2_f32r else F16    # paug dtype must match stage2 lhsT

    with tile.TileContext(nc) as tc:
        with (
            tc.tile_pool(name="apool", bufs=NCH) as apool,
            tc.tile_pool(name="cpool", bufs=NCH) as cpool,
            tc.tile_pool(name="rpool", bufs=NCH) as rpool,
            tc.tile_pool(name="r32pool", bufs=NCH if st2_f32r else 1) as r32pool,
            tc.tile_pool(name="paugpool", bufs=NCH) as paugpool,
            tc.tile_pool(name="gapool", bufs=2) as gapool,
            tc.tile_pool(name="gwpool", bufs=2) as gwpool,
            tc.tile_pool(name="Epool", bufs=2) as Epool,
            tc.tile_pool(name="scrpool", bufs=2) as scrpool,
            tc.tile_pool(name="w2pool", bufs=1) as w2pool,
            tc.tile_pool(name="tiny", bufs=6) as tiny,
            tc.tile_pool(name="ps", bufs=8, space="PSUM") as ps,
        ):
            # ---- DMA triggers are ~0.7us each on a sequencer, so spread them
            # across all five engines; each engine's list is in priority
            # order.  PE start gate: embc half 1 (scalar) + at[0] (tensor).
            hw = NCH * c // 2
            embc_t = cpool.tile([128, NCH * c], F16, tag="c", name="embc")
            at_t = [None] * NCH
            for k in range(NCH):
                at_t[k] = apool.tile([128, D], F16, tag="a", name=f"at{k}")
            embr_t = rpool.tile([128, NCH * R], F16, tag="r", name="embr")
            mrows_t = tiny.tile([4, R], F32R, tag="mr")
            rhs4_t = tiny.tile([4, c], F32R, tag="r4")
            ucol_t = tiny.tile([128, NCH], F32, tag="uc")
            rrow_t = tiny.tile([1, c], F32, tag="rr")
            m0col_t = tiny.tile([128, 1], F32, tag="m0")
            warm_t = tiny.tile([128, 128], F16, tag="warm")

            nc.vector.memset(warm_t[:], 0.0)

            nc.scalar.dma_start(out=at_t[0][:], in_=at_d[0])
            nc.sync.dma_start(out=embc_t[:, 0:hw], in_=embc_d[:, 0:hw])
            nc.gpsimd.dma_start(out=at_t[1][:], in_=at_d[1])
            nc.scalar.dma_start(out=at_t[2][:], in_=at_d[2])
            nc.sync.dma_start(out=embc_t[:, hw:], in_=embc_d[:, hw:])
            nc.gpsimd.dma_start(out=at_t[3][:], in_=at_d[3])
            nc.scalar.dma_start(out=at_t[4][:], in_=at_d[4])
            nc.sync.dma_start(out=at_t[5][:], in_=at_d[5])
            nc.gpsimd.dma_start(out=at_t[6][:], in_=at_d[6])
            nc.scalar.dma_start(out=at_t[7][:], in_=at_d[7])
            nc.gpsimd.dma_start(out=embr_t[:], in_=embr_d)
            nc.sync.dma_start(out=mrows_t[:], in_=mrows_d)
            nc.sync.dma_start(out=rhs4_t[:], in_=rhs4_d)
            nc.sync.dma_start(out=ucol_t[:], in_=ucol_d)
            nc.sync.dma_start(out=rrow_t[:], in_=rrow_d)
            nc.sync.dma_start(out=m0col_t[:], in_=m0col_d)
            embr32_t = None
            if st2_f32r:
                embr32_t = r32pool.tile([128, NCH * R], F32R, tag="r32",
                                        name="embr32")
                nc.sync.dma_start(out=embr32_t[:], in_=embr32_d)

            # ---- PE warmup: ramp the clock out of low p-state while the
            # first input DMAs land; results are discarded.
            warm_ps = ps.tile([128, 512], F32, tag="ps", name="warm_ps")
            for _ in range(12):
                nc.tensor.matmul(warm_ps[:, 0:128], warm_t[:], warm_t[:],
                                 start=True, stop=True)

            # ---- stage 1: P = at.T @ embc  (db-outer over 8 banks)
            st1 = [ps.tile([128, 512], F32, tag="ps", name=f"st1_{da}")
                   for da in range(NCH)]
            for db in range(NCH):
                for da in range(NCH):
                    nc.tensor.matmul(st1[da][:, 0:c],
                                     at_t[db][:, da * 128:(da + 1) * 128],
                                     embc_t[:, db * c:(db + 1) * c],
                                     start=(db == 0), stop=(db == NCH - 1))
            # PSUM -> SBUF with the u-column bias, split across ACT/DVE
            paug = []
            for da in range(NCH):
                pt = paugpool.tile([128, c], pdt, tag="paug")
                if da % 2 == 0:
                    nc.scalar.activation(out=pt[:], in_=st1[da][:, 0:c],
                                         func=AFT.Identity,
                                         bias=ucol_t[:, da:da + 1], scale=1.0)
                else:
                    nc.vector.tensor_scalar_add(pt[:], st1[da][:, 0:c],
                                                ucol_t[:, da:da + 1])
                paug.append(pt)

            # ---- W2 = broadcast of r over cols
            W2 = w2pool.tile([128, c], F32, tag="w2")
            nc.gpsimd.partition_broadcast(W2[:], rrow_t[0:1, :], channels=128)

            # ---- gram -> gw = |G| * r_c  (overlaps the paug copies)
            gw_t = []
            for yc in range(nr):
                Gp = ps.tile([128, 512], F32, tag="ps", name=f"G_{yc}")
                for d2 in range(NCH):
                    nc.tensor.matmul(Gp[:, 0:c],
                                     embr_t[:, d2 * R + yc * 128:
                                            d2 * R + (yc + 1) * 128],
                                     embc_t[:, d2 * c:(d2 + 1) * c],
                                     start=(d2 == 0), stop=(d2 == NCH - 1))
                ga = gapool.tile([128, c], F32, tag="ga")
                nc.scalar.activation(out=ga[:], in_=Gp[:, 0:c], func=AFT.Abs,
                                     bias=0.0, scale=1.0)
                gw = gwpool.tile([128, c], F32, tag="gw")
                nc.vector.tensor_mul(gw[:], ga[:], W2[:])
                gw_t.append(gw)

            # ---- stage 2: L = mask + embr.T @ paug; exp/stt with accums
            ztile = tiny.tile([128, nr], F32, tag="z")
            wtile = tiny.tile([128, nr], F32, tag="w")
            lhs_t = embr32_t if st2_f32r else embr_t
            for yc in range(nr):
                Lp = ps.tile([128, 512], F32, tag="ps", name=f"L_{yc}")
                nc.tensor.matmul(Lp[:, 0:c], mrows_t[:, yc * 128:(yc + 1) * 128],
                                 rhs4_t[:], start=True, stop=False)
                for da in range(NCH):
                    nc.tensor.matmul(Lp[:, 0:c],
                                     lhs_t[:, da * R + yc * 128:
                                           da * R + (yc + 1) * 128],
                                     paug[da][:], start=False, stop=(da == NCH - 1))
                E = Epool.tile([128, c], F32, tag="E")
                nc.scalar.activation(out=E[:], in_=Lp[:, 0:c], func=AFT.Exp,
                                     bias=m0col_t[:], scale=1.0,
                                     accum_out=ztile[:, yc:yc + 1])
                scr = scrpool.tile([128, c], F32, tag="scr")
                nc.vector.scalar_tensor_tensor(
                    out=scr[:], in0=gw_t[yc][:], scalar=1.0, in1=E[:],
                    op0=ALU.mult, op1=ALU.mult,
                    accum_out=wtile[:, yc:yc + 1])

            nc.scalar.dma_start(out=zw_d[:, 0:nr], in_=ztile[:])
            nc.gpsimd.dma_start(out=zw_d[:, nr:2 * nr], in_=wtile[:])

    nc.compile()
    _builds[key] = nc
    return nc


def _pick_pairing(n_rows: np.ndarray, n_cols: np.ndarray):
    """Pair the 16 batches into 8 cores: rows (m1) must fit 2*128 chunks,
    cols (m0) set the free dim; minimize the max col sum."""
    def pairs_from(order):
        return [(int(order[k]), int(order[B - 1 - k])) for k in range(B // 2)]

    best = None
    for key in (-n_cols, -n_rows):
        pr = pairs_from(np.argsort(key, kind="stable"))
        rmax = max(n_rows[a] + n_rows[b] for a, b in pr)
        cmax = max(n_cols[a] + n_cols[b] for a, b in pr)
        cand = (int(np.ceil(max(rmax, 1) / 128)), int(cmax), pr)
        if best is None or (cand[0], cand[1]) < (best[0], best[1]):
            best = cand
    nr, cmax, pr = best
    c = max(256, -(-max(cmax, 1) // 4) * 4)
    return nr, c, pr


def kernel(embeddings, Wq, bq, Wk, bk, attention_masks, token_type_ids):
    global LAST_RESULTS

    emb = np.ascontiguousarray(np.asarray(embeddings, dtype=np.float32))
    Wq = np.asarray(Wq, dtype=np.float32)
    Wk = np.asarray(Wk, dtype=np.float32)
    bq = np.asarray(bq, dtype=np.float32)
    bk = np.asarray(bk, dtype=np.float32)
    am = np.asarray(attention_masks)
    tt = np.asarray(token_type_ids)

    tok = am == 1
    m0 = tok & (tt == 0)   # cols
    m1 = tok & (tt == 1)   # rows
    n_cols = m0.sum(1)
    n_rows = m1.sum(1)

    nr, c, pairing = _pick_pairing(n_rows, n_cols)
    R = nr * 128
    nc = _build(nr, c, ST2_F32R)

    # ---- host constant folding
    Wq64, Wk64 = Wq.astype(np.float64), Wk.astype(np.float64)
    A = (Wq64.T @ Wk64).astype(np.float32)          # [db, da] stage-1 lhsT
    at16 = _f16(A).reshape(NCH, 128, D)
    u = (Wk64.T @ bq.astype(np.float64)).astype(np.float32)       # P bias
    ucol = np.ascontiguousarray(u.reshape(NCH, 128).T)            # [128, NCH]
    u2 = Wq64.T @ bk.astype(np.float64)             # prow direction
    c0 = float(bq.astype(np.float64) @ bk.astype(np.float64))

    nrm = np.sqrt(np.einsum("bsd,bsd->bs", emb, emb, dtype=np.float64))
    rr_full = (1.0 / np.maximum(nrm, EPS)).astype(np.float32)     # [B, S]

    in_maps = []
    row_meta = []   # per core: (b0, nrow0, b1, nrow1, r_rows[R])
    for (b0, b1) in pairing:
        ridx = [(b, j) for b in (b0, b1) for j in np.nonzero(m1[b])[0]]
        cidx = [(b, j) for b in (b0, b1) for j in np.nonzero(m0[b])[0]]
        nrow0 = int(n_rows[b0])
        ncol0 = int(n_cols[b0])
        nrow = len(ridx)
        ncol = len(cidx)

        er = np.zeros((R, D), np.float32)
        for i, (b, j) in enumerate(ridx):
            er[i] = emb[b, j]
        ec = np.zeros((c, D), np.float32)
        for i, (b, j) in enumerate(cidx):
            ec[i] = emb[b, j]

        # pack [tok, D] -> [128, NCH*n]: line p holds chunk-major columns,
        # chunk k at cols [k*n, (k+1)*n), partition p <-> d = k*128+p
        erw = er.T.reshape(NCH, 128, R).transpose(1, 0, 2).reshape(128, NCH * R)
        ecw = ec.T.reshape(NCH, 128, c).transpose(1, 0, 2).reshape(128, NCH * c)
        embr = _f16(erw)
        embc = _f16(ecw)

        prow = (ec.astype(np.float64) @ u2 + c0).astype(np.float32)
        prow[ncol:] = NEG                       # padded cols masked via row0

        mrows = np.zeros((4, R), np.float32)
        mrows[0, :] = 1.0
        mrows[1, :nrow0] = 1.0                  # b0 rows
        mrows[2, nrow0:nrow] = 1.0              # b1 rows
        mrows[3, nrow:] = 1.0                   # padded rows
        rhs4 = np.zeros((4, c), np.float32)
        rhs4[0] = prow
        rhs4[1, ncol0:ncol] = NEG               # b1 cols, masked for b0 rows
        rhs4[2, :ncol0] = NEG                   # b0 cols, masked for b1 rows
        rhs4[3, :] = NEG                        # all cols, masked for pad rows

        rrow = np.zeros((1, c), np.float32)
        r_cols = np.array([rr_full[b, j] for (b, j) in cidx], np.float32)
        rrow[0, :ncol] = r_cols
        r_rows = np.zeros(R, np.float32)
        r_rows[:nrow] = [rr_full[b, j] for (b, j) in ridx]

        im = {
            "at": at16,
            "embc": embc,
            "embr": embr,
            "mrows": _to_fp32r(mrows),
            "rhs4": _to_fp32r(rhs4),
            "ucol": ucol,
            "rrow": rrow,
            "m0col": np.full((128, 1), -M0, np.float32),
        }
        if ST2_F32R:
            im["embr32"] = _to_fp32r(erw)
        in_maps.append(im)
        row_meta.append((b0, nrow0, b1, nrow - nrow0))

    valid = m0.any(axis=1) & m1.any(axis=1)
    for attempt in range(3):
        res = run_bass_kernel_spmd(nc, in_maps, core_ids=list(range(NCORES)),
                                   trace=PROFILE)
        LAST_RESULTS = res
        ok = all(np.isfinite(res.results[i]["zw"]).all() for i in range(NCORES))
        if ok:
            break
        for im in in_maps:    # overflow escape hatch: larger shift, no recompile
            im["m0col"] = im["m0col"] * 4.0

    cs = np.zeros(B, np.float64)
    for i in range(NCORES):
        zw = res.results[i]["zw"].astype(np.float64)      # [128, 2*nr]
        zflat = zw[:, 0:nr].T.ravel()                     # row-major [R]
        wflat = zw[:, nr:2 * nr].T.ravel()
        b0, nrow0, b1, nrow1 = row_meta[i]
        r_rows = np.zeros(R, np.float64)
        ridx = [(b, j) for b in (b0, b1) for j in np.nonzero(m1[b])[0]]
        r_rows[:len(ridx)] = [rr_full[b, j] for (b, j) in ridx]
        wr = wflat * r_rows
        if valid[b0]:
            z = zflat[:nrow0].sum()
            cs[b0] = wr[:nrow0].sum() / (z + 1e-300)
        if valid[b1]:
            z = zflat[nrow0:nrow0 + nrow1].sum()
            cs[b1] = wr[nrow0:nrow0 + nrow1].sum() / (z + 1e-300)
    return cs.astype(np.float32)
